# revision 34
# baseline (speedup 1.0000x reference)
"""Trainium2 Bass kernel for nn_CustomCrossAttention_21406117003981.

Full inputs in, full output out. Data-parallel over batch: 16 batches ->
8 cores x 2 batches. The host precomputes everything that is O(B*T*D) or
smaller and outside the attention quadratic: the Gaussian resample
(eps from the reference's fixed PRNG keys, per-batch mean / covariance /
Cholesky, K = mu + eps @ L^T), the transposed layouts, the gate row-dots
A@wA1 / V@wV1, and the cosine gate alpha = sigmoid(cos(A, V)). The device
kernel does the dominant O(B*T^2*D) work: both cross-attention directions
(QK^T, softmax, PV) plus the per-row gated fusion and final combine.

Per local batch i (core c handles global batches 2c, 2c+1):
  at/vt/kvt/kat [256, 2048]  f32r : A^T, V^T, K_v^T, K_a^T
  aext/vext     [2048, 260]  fp16 : [X | gdot_hi | gdot_lo | ones | 0]
  alp/agl/vgl   [2048]       f32  : alpha, A@wA1, V@wV1

Attention is computed transposed: S^T = K @ Q^T so that P^T = exp(S^T/16)
is directly the lhsT of the PV matmul; the ext "ones" column yields the
softmax row-sum and the gate hi/lo columns yield att @ w_gate through the
same matmul at ~fp32 accuracy. Softmax skips max-subtraction (scores are
O(6) for this data; exp cannot overflow and the result is mathematically
identical). The va direction is consumed tile-by-tile straight from PSUM
into the final combine so the kernel tail stays short.
"""

import os

import numpy as np

B, T, D = 16, 2048, 256
NCORES = 8
BPC = B // NCORES  # batches per core
DE = D + 4  # ext width: values, gate-dot hi, gate-dot lo, ones, pad
GHI, GLO, ONES_C = D, D + 1, D + 2
QT = T // 128  # 16 q-tiles per batch
KT = T // 128  # 16 k-tiles

# knobs (env-overridable for experiments; defaults = shipped config)
MM_DTYPE = os.environ.get("CCA_MM_DTYPE", "f32r")  # f32 | f32r
P_DT = os.environ.get("CCA_P_DT", "fp16")  # fp16 | f32 : P^T / PV-values dtype
CHUNK = int(os.environ.get("CCA_CHUNK", "512"))  # score-chunk width (multiple of 512)
GP = os.environ.get("CCA_GP", "1") == "1"  # offload part of combine to gpsimd

_cache: dict = {}


def _build():
    import concourse.bacc as bacc
    import concourse.mybir as mybir
    import concourse.tile as tile

    dt = mybir.dt
    f32 = dt.float32
    AF = mybir.ActivationFunctionType
    OP = mybir.AluOpType

    nc = bacc.Bacc("TRN2", target_bir_lowering=False, debug=False)

    mdt = dt.float32r if MM_DTYPE == "f32r" else f32
    edt = dt.float16 if P_DT == "fp16" else f32

    ins = {}
    for i in range(BPC):
        for nm in ("at", "vt", "kvt", "kat"):
            ins[f"{nm}{i}"] = nc.dram_tensor(f"{nm}{i}", [D, T], mdt, kind="ExternalInput")
        for nm in ("aext", "vext"):
            ins[f"{nm}{i}"] = nc.dram_tensor(f"{nm}{i}", [T, DE], edt, kind="ExternalInput")
        for nm in ("alp", "agl", "vgl"):
            ins[f"{nm}{i}"] = nc.dram_tensor(f"{nm}{i}", [T], f32, kind="ExternalInput")
    bg_d = nc.dram_tensor("bg", [1, 2], f32, kind="ExternalInput")
    outs = [nc.dram_tensor(f"out{i}", [T, D], f32, kind="ExternalOutput") for i in range(BPC)]

    nq = CHUNK // 128  # q-tiles per score chunk
    ngr = QT // nq  # chunks per direction

    with tile.TileContext(nc) as tc:
        from contextlib import ExitStack

        with ExitStack() as ctx:
            consts = ctx.enter_context(tc.tile_pool(name="consts", bufs=1))
            kq = ctx.enter_context(tc.tile_pool(name="kq", bufs=4))
            ext = ctx.enter_context(tc.tile_pool(name="ext", bufs=3))
            attsb = ctx.enter_context(tc.tile_pool(name="attsb", bufs=3))
            ptp = ctx.enter_context(tc.tile_pool(name="ptp", bufs=KT + 2))
            sm = ctx.enter_context(tc.tile_pool(name="sm", bufs=30))
            outp = ctx.enter_context(tc.tile_pool(name="outp", bufs=6))
            ps_s = ctx.enter_context(tc.tile_pool(name="ps_s", bufs=2, space="PSUM"))
            ps_att = ctx.enter_context(tc.tile_pool(name="ps_att", bufs=4, space="PSUM"))

            # constants: negated gate biases as per-partition columns
            bgt = consts.tile([128, 2], f32)
            nc.sync.dma_start(out=bgt, in_=bg_d.ap().to_broadcast([128, 2]))
            nbg = consts.tile([128, 2], f32)
            nc.vector.tensor_scalar_mul(nbg, bgt, -1.0)

            for i in range(BPC):
                # ---- per-batch loads ----
                # kvt/at split by d-half so the first scores matmul can start
                # after ~half the load latency
                at_t = kq.tile([128, 2, T], mdt, tag="kq", name=f"at_t{i}")
                at_src = ins[f"at{i}"].ap().rearrange("(h p) t -> p h t", p=128)
                kvt_t = kq.tile([128, 2, T], mdt, tag="kq", name=f"kvt_t{i}")
                kvt_src = ins[f"kvt{i}"].ap().rearrange("(h p) t -> p h t", p=128)
                for h in range(2):
                    nc.sync.dma_start(out=kvt_t[:, h, :], in_=kvt_src[:, h, :])
                    nc.gpsimd.dma_start(out=at_t[:, h, :], in_=at_src[:, h, :])
                vext_t = ext.tile([128, QT, DE], edt, tag="ext", name=f"vext_t{i}")
                nc.sync.dma_start(out=vext_t, in_=ins[f"vext{i}"].ap().rearrange("(n p) c -> p n c", p=128))
                aext_t = ext.tile([128, QT, DE], edt, tag="ext", name=f"aext_t{i}")
                nc.gpsimd.dma_start(out=aext_t, in_=ins[f"aext{i}"].ap().rearrange("(n p) c -> p n c", p=128))
                vt_t = kq.tile([128, 2, T], mdt, tag="kq", name=f"vt_t{i}")
                nc.sync.dma_start(out=vt_t, in_=ins[f"vt{i}"].ap().rearrange("(h p) t -> p h t", p=128))
                kat_t = kq.tile([128, 2, T], mdt, tag="kq", name=f"kat_t{i}")
                nc.gpsimd.dma_start(out=kat_t, in_=ins[f"kat{i}"].ap().rearrange("(h p) t -> p h t", p=128))

                alp = sm.tile([128, QT], f32, tag="smt", name=f"alp{i}")
                nc.sync.dma_start(out=alp, in_=ins[f"alp{i}"].ap().rearrange("(n p) -> p n", p=128))
                agl = sm.tile([128, QT], f32, tag="smt", name=f"agl{i}")
                nc.sync.dma_start(out=agl, in_=ins[f"agl{i}"].ap().rearrange("(n p) -> p n", p=128))
                vgl = sm.tile([128, QT], f32, tag="smt", name=f"vgl{i}")
                nc.sync.dma_start(out=vgl, in_=ins[f"vgl{i}"].ap().rearrange("(n p) -> p n", p=128))
                am1 = sm.tile([128, QT], f32, tag="smt", name=f"am1{i}")
                nc.vector.tensor_scalar(am1, alp, -1.0, 1.0, OP.mult, OP.add)

                def scores_chunk(kt_src, qt_src, qp, tag):
                    # P^T tiles for q-chunk qp: exp(K^T[d,k].T @ Q^T[d,qchunk] / 16)
                    pts = []
                    for k in range(KT):
                        ps = ps_s.tile([128, CHUNK], f32, tag="ps", name=f"ps_{tag}_{qp}_{k}")
                        for h in range(2):
                            for qq in range(CHUNK // 512):
                                nc.tensor.matmul(
                                    ps[:, qq * 512 : (qq + 1) * 512],
                                    lhsT=kt_src[:, h, k * 128 : (k + 1) * 128],
                                    rhs=qt_src[:, h, qp * CHUNK + qq * 512 : qp * CHUNK + (qq + 1) * 512],
                                    start=(h == 0),
                                    stop=(h == 1),
                                )
                        pt = ptp.tile([128, CHUNK], edt, tag="pt", name=f"pt_{tag}_{qp}_{k}")
                        for qq in range(CHUNK // 512):
                            nc.scalar.activation(
                                pt[:, qq * 512 : (qq + 1) * 512],
                                ps[:, qq * 512 : (qq + 1) * 512],
                                AF.Exp,
                                scale=1.0 / 16.0,
                            )
                        pts.append(pt)
                    return pts

                # ---- direction av: queries A, keys K_v, values V ----
                att_av = attsb.tile([128, QT, DE], f32, tag="attsb", name=f"att_av{i}")
                for qp in range(ngr):
                    pts = scores_chunk(kvt_t, at_t, qp, f"av{i}")
                    for j in range(nq):
                        q = qp * nq + j
                        pa = ps_att.tile([128, DE], f32, tag="pa", name=f"paav{i}_{q}")
                        for k in range(KT):
                            nc.tensor.matmul(
                                pa,
                                lhsT=pts[k][:, j * 128 : (j + 1) * 128],
                                rhs=vext_t[:, k, :],
                                start=(k == 0),
                                stop=(k == KT - 1),
                            )
                        nc.vector.tensor_copy(att_av[:, q, :], pa)

                # batched av-side row quantities [128, QT]
                def smt(nm):
                    return sm.tile([128, QT], f32, tag="smt", name=f"{nm}{i}")

                r_av, gA, c1, c2 = smt("r_av"), smt("gA"), smt("c1"), smt("c2")
                nc.vector.reciprocal(r_av, att_av[:, :, ONES_C : ONES_C + 1].squeeze())
                t0 = smt("t0")
                nc.vector.tensor_add(
                    t0, att_av[:, :, GHI : GHI + 1].squeeze(), att_av[:, :, GLO : GLO + 1].squeeze()
                )
                nc.vector.tensor_mul(t0, t0, r_av)
                nc.vector.tensor_add(t0, t0, agl)
                e0 = smt("e0")
                nc.scalar.activation(e0, t0, AF.Exp, bias=nbg[:, 0:1], scale=-1.0)
                nc.vector.tensor_scalar_add(e0, e0, 1.0)
                nc.vector.reciprocal(gA, e0)
                nc.vector.tensor_mul(c1, alp, gA)
                t1 = smt("t1")
                nc.vector.tensor_sub(t1, alp, c1)
                nc.vector.tensor_mul(c2, t1, r_av)

                # ---- direction va: queries V, keys K_a, values A ----
                # evacuated per tile, combined per chunk (keeps the kernel
                # tail short while PSUM slots recycle immediately)
                att_va = attsb.tile([128, QT, DE], f32, tag="attsb", name=f"att_va{i}")
                outv = outs[i].ap().rearrange("(n p) c -> p n c", p=128)
                for qp in range(ngr):
                    pts = scores_chunk(kat_t, vt_t, qp, f"va{i}")
                    for j in range(nq):
                        q = qp * nq + j
                        pa = ps_att.tile([128, DE], f32, tag="pa", name=f"pava{i}_{q}")
                        for k in range(KT):
                            nc.tensor.matmul(
                                pa,
                                lhsT=pts[k][:, j * 128 : (j + 1) * 128],
                                rhs=aext_t[:, k, :],
                                start=(k == 0),
                                stop=(k == KT - 1),
                            )
                        nc.vector.tensor_copy(att_va[:, q, :], pa)

                    # batched row quantities, in waves of 4 q-tiles so the
                    # last wave's combine overlaps the remaining PV matmuls
                    for wv in range(nq // 4):
                        qs = slice(qp * nq + wv * 4, qp * nq + (wv + 1) * 4)
                        rv = sm.tile([128, 4, 4], f32, tag="rv", name=f"rv{i}_{qp}_{wv}")
                        nc.vector.reciprocal(rv[:, :, 0], att_va[:, qs, ONES_C : ONES_C + 1].squeeze())
                        nc.vector.tensor_add(
                            rv[:, :, 1], att_va[:, qs, GHI : GHI + 1].squeeze(),
                            att_va[:, qs, GLO : GLO + 1].squeeze(),
                        )
                        nc.vector.tensor_mul(rv[:, :, 1], rv[:, :, 1], rv[:, :, 0])
                        nc.vector.tensor_add(rv[:, :, 1], rv[:, :, 1], vgl[:, qs])
                        nc.scalar.activation(rv[:, :, 2], rv[:, :, 1], AF.Exp, bias=nbg[:, 1:2], scale=-1.0)
                        nc.vector.tensor_scalar_add(rv[:, :, 2], rv[:, :, 2], 1.0)
                        nc.vector.reciprocal(rv[:, :, 2], rv[:, :, 2])  # g_V
                        nc.vector.tensor_mul(rv[:, :, 2], rv[:, :, 2], am1[:, qs])  # c3
                        nc.vector.tensor_sub(rv[:, :, 3], am1[:, qs], rv[:, :, 2])
                        nc.vector.tensor_mul(rv[:, :, 3], rv[:, :, 3], rv[:, :, 0])  # c4

                        for j in range(4):
                            q = qp * nq + wv * 4 + j
                            # combine: c1*A + c2*att_av + c3*V + c4*att_va
                            o1 = outp.tile([128, D], f32, tag="o1", name=f"o1_{i}_{q}")
                            t_a = outp.tile([128, D], f32, tag="ta", name=f"ta_{i}_{q}")
                            nc.vector.tensor_scalar_mul(o1, aext_t[:, q, 0:D], c1[:, q : q + 1])
                            nc.vector.tensor_scalar_mul(t_a, att_av[:, q, 0:D], c2[:, q : q + 1])
                            nc.vector.tensor_add(o1, o1, t_a)
                            t_b = outp.tile([128, D], f32, tag="tb", name=f"tb_{i}_{q}")
                            nc.vector.tensor_scalar_mul(t_b, vext_t[:, q, 0:D], rv[:, j, 2:3])
                            t_c = outp.tile([128, D], f32, tag="tc", name=f"tc_{i}_{q}")
                            nc.vector.tensor_scalar_mul(t_c, att_va[:, q, 0:D], rv[:, j, 3:4])
                            nc.vector.tensor_add(t_b, t_b, t_c)
                            nc.vector.tensor_add(o1, o1, t_b)
                            nc.sync.dma_start(out=outv[:, q, :], in_=o1)

    nc.compile()
    return nc


def _host_prep(A, V, W_A_g, W_V_g, b_A_g, b_V_g):
    import jax

    if "eps" not in _cache:
        fn = jax.jit(
            lambda: (
                jax.random.normal(jax.random.key(42), (B, T, D), dtype=np.float32),
                jax.random.normal(jax.random.key(43), (B, T, D), dtype=np.float32),
            ),
            backend="cpu",
        )
        ev, ea = fn()
        _cache["eps"] = (np.asarray(ev), np.asarray(ea))
    eps_v, eps_a = _cache["eps"]

    def resample_T(X, eps):
        mu = X.mean(axis=1, dtype=np.float32)
        Xc = X - mu[:, None, :]
        Sigma = np.matmul(Xc.transpose(0, 2, 1), Xc) / np.float32(T - 1)
        Sigma = Sigma + (1e-6 * np.eye(D, dtype=np.float32))[None]
        L = np.linalg.cholesky(Sigma).astype(np.float32)
        KTr = np.matmul(L, eps.transpose(0, 2, 1)) + mu[:, :, None]
        return np.ascontiguousarray(KTr.astype(np.float32))  # [B, D, T]

    KvT = resample_T(V, eps_v)
    KaT = resample_T(A, eps_a)

    wA1, wA2 = W_A_g[0, :D], W_A_g[0, D:]
    wV1, wV2 = W_V_g[0, :D], W_V_g[0, D:]
    ones = np.ones((B, T, 1), np.float32)
    v_g = (V @ wA2)[..., None]
    a_g = (A @ wV2)[..., None]

    if P_DT == "fp16":
        vg_hi = v_g.astype(np.float16).astype(np.float32)
        ag_hi = a_g.astype(np.float16).astype(np.float32)
        zeros = np.zeros((B, T, 1), np.float32)
        vext = np.concatenate([V, vg_hi, v_g - vg_hi, ones, zeros], axis=2).astype(np.float16)
        aext = np.concatenate([A, ag_hi, a_g - ag_hi, ones, zeros], axis=2).astype(np.float16)
    else:
        zeros = np.zeros((B, T, 1), np.float32)
        vext = np.concatenate([V, v_g, zeros, ones, zeros], axis=2).astype(np.float32)
        aext = np.concatenate([A, a_g, zeros, ones, zeros], axis=2).astype(np.float32)

    # host-side row stats: gate row-dots and the cosine gate alpha
    a_glin = np.einsum("btd,d->bt", A, wA1).astype(np.float32)
    v_glin = np.einsum("btd,d->bt", V, wV1).astype(np.float32)
    dot = np.sum(A * V, axis=2)
    nrm = np.sqrt(np.sum(A * A, axis=2)) * np.sqrt(np.sum(V * V, axis=2))
    cos = dot / np.maximum(nrm, 1e-8)
    alpha = (1.0 / (1.0 + np.exp(-cos))).astype(np.float32)

    bg = np.array([[b_A_g[0], b_V_g[0]]], dtype=np.float32)

    AT = np.ascontiguousarray(A.transpose(0, 2, 1))
    VT = np.ascontiguousarray(V.transpose(0, 2, 1))
    vext = np.ascontiguousarray(vext)
    aext = np.ascontiguousarray(aext)

    in_maps = []
    for c in range(NCORES):
        m = {"bg": bg}
        for i in range(BPC):
            b = c * BPC + i
            m[f"at{i}"] = AT[b]
            m[f"vt{i}"] = VT[b]
            m[f"kvt{i}"] = KvT[b]
            m[f"kat{i}"] = KaT[b]
            m[f"aext{i}"] = aext[b]
            m[f"vext{i}"] = vext[b]
            m[f"alp{i}"] = np.ascontiguousarray(alpha[b])
            m[f"agl{i}"] = np.ascontiguousarray(a_glin[b])
            m[f"vgl{i}"] = np.ascontiguousarray(v_glin[b])
        in_maps.append(m)
    return in_maps


def kernel(A, V, W_A_g, W_V_g, b_A_g, b_V_g):
    from concourse import bass_utils

    A = np.asarray(A, dtype=np.float32)
    V = np.asarray(V, dtype=np.float32)
    W_A_g = np.asarray(W_A_g, dtype=np.float32)
    W_V_g = np.asarray(W_V_g, dtype=np.float32)
    b_A_g = np.asarray(b_A_g, dtype=np.float32)
    b_V_g = np.asarray(b_V_g, dtype=np.float32)

    if "nc" not in _cache:
        _cache["nc"] = _build()
    nc = _cache["nc"]

    in_maps = _host_prep(A, V, W_A_g, W_V_g, b_A_g, b_V_g)
    res = bass_utils.run_bass_kernel_spmd(nc, in_maps, core_ids=list(range(NCORES)))

    out = np.empty((B, T, D), np.float32)
    for c in range(NCORES):
        for i in range(BPC):
            out[c * BPC + i] = res.results[c][f"out{i}"]
    return out


# revision 35
# speedup vs baseline: 1.0718x; 1.0718x over previous
"""Trainium2 Bass kernel for nn_CustomCrossAttention_21406117003981.

Full inputs in, full output out. Data-parallel over batch: 16 batches ->
8 cores x 2 batches. The host precomputes everything that is O(B*T*D) or
smaller and outside the attention quadratic: the Gaussian resample
(eps from the reference's fixed PRNG keys, per-batch mean / covariance /
Cholesky, K = mu + eps @ L^T), the transposed layouts, the gate row-dots
A@wA1 / V@wV1, and the cosine gate alpha = sigmoid(cos(A, V)). The device
kernel does the dominant O(B*T^2*D) work: both cross-attention directions
(QK^T, softmax, PV) plus the per-row gated fusion and final combine.

Per local batch i (core c handles global batches 2c, 2c+1):
  at/vt/kvt/kat [256, 2048]  f32r : A^T, V^T, K_v^T, K_a^T
  aext/vext     [2048, 260]  fp16 : [X | gdot_hi | gdot_lo | ones | 0]
  alp/agl/vgl   [2048]       f32  : alpha, A@wA1, V@wV1

Attention is computed transposed: S^T = K @ Q^T so that P^T = exp(S^T/16)
is directly the lhsT of the PV matmul; the ext "ones" column yields the
softmax row-sum and the gate hi/lo columns yield att @ w_gate through the
same matmul at ~fp32 accuracy. Softmax skips max-subtraction (scores are
O(6) for this data; exp cannot overflow and the result is mathematically
identical). The va direction is consumed tile-by-tile straight from PSUM
into the final combine so the kernel tail stays short.
"""

import os

import numpy as np

B, T, D = 16, 2048, 256
NCORES = 8
BPC = B // NCORES  # batches per core
DE = D + 4  # ext width: values, gate-dot hi, gate-dot lo, ones, pad
GHI, GLO, ONES_C = D, D + 1, D + 2
QT = T // 128  # 16 q-tiles per batch
KT = T // 128  # 16 k-tiles

# knobs (env-overridable for experiments; defaults = shipped config)
MM_DTYPE = os.environ.get("CCA_MM_DTYPE", "f32r")  # f32 | f32r
P_DT = os.environ.get("CCA_P_DT", "fp16")  # fp16 | f32 : P^T / PV-values dtype
CHUNK = int(os.environ.get("CCA_CHUNK", "512"))  # score-chunk width (multiple of 512)
GP = os.environ.get("CCA_GP", "1") == "1"  # offload part of combine to gpsimd

_cache: dict = {}


def _build():
    import concourse.bacc as bacc
    import concourse.mybir as mybir
    import concourse.tile as tile

    dt = mybir.dt
    f32 = dt.float32
    AF = mybir.ActivationFunctionType
    OP = mybir.AluOpType

    nc = bacc.Bacc("TRN2", target_bir_lowering=False, debug=False)

    mdt = dt.float32r if MM_DTYPE == "f32r" else f32
    edt = dt.float16 if P_DT == "fp16" else f32

    ins = {}
    for i in range(BPC):
        for nm in ("at", "vt", "kvt", "kat"):
            ins[f"{nm}{i}"] = nc.dram_tensor(f"{nm}{i}", [D, T], mdt, kind="ExternalInput")
        for nm in ("aext", "vext"):
            ins[f"{nm}{i}"] = nc.dram_tensor(f"{nm}{i}", [T, DE], edt, kind="ExternalInput")
        for nm in ("alp", "agl", "vgl"):
            ins[f"{nm}{i}"] = nc.dram_tensor(f"{nm}{i}", [T], f32, kind="ExternalInput")
    bg_d = nc.dram_tensor("bg", [1, 2], f32, kind="ExternalInput")
    outs = [nc.dram_tensor(f"out{i}", [T, D], f32, kind="ExternalOutput") for i in range(BPC)]

    nq = CHUNK // 128  # q-tiles per score chunk
    ngr = QT // nq  # chunks per direction

    with tile.TileContext(nc) as tc:
        from contextlib import ExitStack

        with ExitStack() as ctx:
            consts = ctx.enter_context(tc.tile_pool(name="consts", bufs=1))
            kq = ctx.enter_context(tc.tile_pool(name="kq", bufs=4))
            ext = ctx.enter_context(tc.tile_pool(name="ext", bufs=3))
            attsb = ctx.enter_context(tc.tile_pool(name="attsb", bufs=3))
            ptp = ctx.enter_context(tc.tile_pool(name="ptp", bufs=KT + 2))
            sm = ctx.enter_context(tc.tile_pool(name="sm", bufs=30))
            outp = ctx.enter_context(tc.tile_pool(name="outp", bufs=6))
            ps_s = ctx.enter_context(tc.tile_pool(name="ps_s", bufs=2, space="PSUM"))
            ps_att = ctx.enter_context(tc.tile_pool(name="ps_att", bufs=4, space="PSUM"))

            # constants: negated gate biases as per-partition columns
            bgt = consts.tile([128, 2], f32)
            nc.sync.dma_start(out=bgt, in_=bg_d.ap().to_broadcast([128, 2]))
            nbg = consts.tile([128, 2], f32)
            nc.vector.tensor_scalar_mul(nbg, bgt, -1.0)

            for i in range(BPC):
                # ---- per-batch loads ----
                # kvt/at split by d-half so the first scores matmul can start
                # after ~half the load latency
                at_t = kq.tile([128, 2, T], mdt, tag="kq", name=f"at_t{i}")
                at_src = ins[f"at{i}"].ap().rearrange("(h p) t -> p h t", p=128)
                kvt_t = kq.tile([128, 2, T], mdt, tag="kq", name=f"kvt_t{i}")
                kvt_src = ins[f"kvt{i}"].ap().rearrange("(h p) t -> p h t", p=128)
                for h in range(2):
                    nc.sync.dma_start(out=kvt_t[:, h, :], in_=kvt_src[:, h, :])
                    nc.sync.dma_start(out=at_t[:, h, :], in_=at_src[:, h, :])
                vext_t = ext.tile([128, QT, DE], edt, tag="ext", name=f"vext_t{i}")
                nc.sync.dma_start(out=vext_t, in_=ins[f"vext{i}"].ap().rearrange("(n p) c -> p n c", p=128))
                aext_t = ext.tile([128, QT, DE], edt, tag="ext", name=f"aext_t{i}")
                nc.sync.dma_start(out=aext_t, in_=ins[f"aext{i}"].ap().rearrange("(n p) c -> p n c", p=128))
                vt_t = kq.tile([128, 2, T], mdt, tag="kq", name=f"vt_t{i}")
                nc.sync.dma_start(out=vt_t, in_=ins[f"vt{i}"].ap().rearrange("(h p) t -> p h t", p=128))
                kat_t = kq.tile([128, 2, T], mdt, tag="kq", name=f"kat_t{i}")
                nc.sync.dma_start(out=kat_t, in_=ins[f"kat{i}"].ap().rearrange("(h p) t -> p h t", p=128))

                alp = sm.tile([128, QT], f32, tag="smt", name=f"alp{i}")
                nc.sync.dma_start(out=alp, in_=ins[f"alp{i}"].ap().rearrange("(n p) -> p n", p=128))
                agl = sm.tile([128, QT], f32, tag="smt", name=f"agl{i}")
                nc.sync.dma_start(out=agl, in_=ins[f"agl{i}"].ap().rearrange("(n p) -> p n", p=128))
                vgl = sm.tile([128, QT], f32, tag="smt", name=f"vgl{i}")
                nc.sync.dma_start(out=vgl, in_=ins[f"vgl{i}"].ap().rearrange("(n p) -> p n", p=128))
                am1 = sm.tile([128, QT], f32, tag="smt", name=f"am1{i}")
                nc.vector.tensor_scalar(am1, alp, -1.0, 1.0, OP.mult, OP.add)

                def scores_chunk(kt_src, qt_src, qp, tag):
                    # P^T tiles for q-chunk qp: exp(K^T[d,k].T @ Q^T[d,qchunk] / 16)
                    pts = []
                    for k in range(KT):
                        ps = ps_s.tile([128, CHUNK], f32, tag="ps", name=f"ps_{tag}_{qp}_{k}")
                        for h in range(2):
                            for qq in range(CHUNK // 512):
                                nc.tensor.matmul(
                                    ps[:, qq * 512 : (qq + 1) * 512],
                                    lhsT=kt_src[:, h, k * 128 : (k + 1) * 128],
                                    rhs=qt_src[:, h, qp * CHUNK + qq * 512 : qp * CHUNK + (qq + 1) * 512],
                                    start=(h == 0),
                                    stop=(h == 1),
                                )
                        pt = ptp.tile([128, CHUNK], edt, tag="pt", name=f"pt_{tag}_{qp}_{k}")
                        for qq in range(CHUNK // 512):
                            nc.scalar.activation(
                                pt[:, qq * 512 : (qq + 1) * 512],
                                ps[:, qq * 512 : (qq + 1) * 512],
                                AF.Exp,
                                scale=1.0 / 16.0,
                            )
                        pts.append(pt)
                    return pts

                # ---- direction av: queries A, keys K_v, values V ----
                att_av = attsb.tile([128, QT, DE], f32, tag="attsb", name=f"att_av{i}")
                for qp in range(ngr):
                    pts = scores_chunk(kvt_t, at_t, qp, f"av{i}")
                    for j in range(nq):
                        q = qp * nq + j
                        pa = ps_att.tile([128, DE], f32, tag="pa", name=f"paav{i}_{q}")
                        for k in range(KT):
                            nc.tensor.matmul(
                                pa,
                                lhsT=pts[k][:, j * 128 : (j + 1) * 128],
                                rhs=vext_t[:, k, :],
                                start=(k == 0),
                                stop=(k == KT - 1),
                            )
                        nc.vector.tensor_copy(att_av[:, q, :], pa)

                # batched av-side row quantities [128, QT]
                def smt(nm):
                    return sm.tile([128, QT], f32, tag="smt", name=f"{nm}{i}")

                r_av, gA, c1, c2 = smt("r_av"), smt("gA"), smt("c1"), smt("c2")
                nc.vector.reciprocal(r_av, att_av[:, :, ONES_C : ONES_C + 1].squeeze())
                t0 = smt("t0")
                nc.vector.tensor_add(
                    t0, att_av[:, :, GHI : GHI + 1].squeeze(), att_av[:, :, GLO : GLO + 1].squeeze()
                )
                nc.vector.tensor_mul(t0, t0, r_av)
                nc.vector.tensor_add(t0, t0, agl)
                e0 = smt("e0")
                nc.scalar.activation(e0, t0, AF.Exp, bias=nbg[:, 0:1], scale=-1.0)
                nc.vector.tensor_scalar_add(e0, e0, 1.0)
                nc.vector.reciprocal(gA, e0)
                nc.vector.tensor_mul(c1, alp, gA)
                t1 = smt("t1")
                nc.vector.tensor_sub(t1, alp, c1)
                nc.vector.tensor_mul(c2, t1, r_av)

                # ---- direction va: queries V, keys K_a, values A ----
                # evacuated per tile, combined per chunk (keeps the kernel
                # tail short while PSUM slots recycle immediately)
                att_va = attsb.tile([128, QT, DE], f32, tag="attsb", name=f"att_va{i}")
                outv = outs[i].ap().rearrange("(n p) c -> p n c", p=128)
                for qp in range(ngr):
                    pts = scores_chunk(kat_t, vt_t, qp, f"va{i}")
                    for j in range(nq):
                        q = qp * nq + j
                        pa = ps_att.tile([128, DE], f32, tag="pa", name=f"pava{i}_{q}")
                        for k in range(KT):
                            nc.tensor.matmul(
                                pa,
                                lhsT=pts[k][:, j * 128 : (j + 1) * 128],
                                rhs=aext_t[:, k, :],
                                start=(k == 0),
                                stop=(k == KT - 1),
                            )
                        nc.vector.tensor_copy(att_va[:, q, :], pa)

                    # batched row quantities, in waves of 4 q-tiles so the
                    # last wave's combine overlaps the remaining PV matmuls
                    for wv in range(nq // 4):
                        qs = slice(qp * nq + wv * 4, qp * nq + (wv + 1) * 4)
                        rv = sm.tile([128, 4, 4], f32, tag="rv", name=f"rv{i}_{qp}_{wv}")
                        nc.vector.reciprocal(rv[:, :, 0], att_va[:, qs, ONES_C : ONES_C + 1].squeeze())
                        nc.vector.tensor_add(
                            rv[:, :, 1], att_va[:, qs, GHI : GHI + 1].squeeze(),
                            att_va[:, qs, GLO : GLO + 1].squeeze(),
                        )
                        nc.vector.tensor_mul(rv[:, :, 1], rv[:, :, 1], rv[:, :, 0])
                        nc.vector.tensor_add(rv[:, :, 1], rv[:, :, 1], vgl[:, qs])
                        nc.scalar.activation(rv[:, :, 2], rv[:, :, 1], AF.Exp, bias=nbg[:, 1:2], scale=-1.0)
                        nc.vector.tensor_scalar_add(rv[:, :, 2], rv[:, :, 2], 1.0)
                        nc.vector.reciprocal(rv[:, :, 2], rv[:, :, 2])  # g_V
                        nc.vector.tensor_mul(rv[:, :, 2], rv[:, :, 2], am1[:, qs])  # c3
                        nc.vector.tensor_sub(rv[:, :, 3], am1[:, qs], rv[:, :, 2])
                        nc.vector.tensor_mul(rv[:, :, 3], rv[:, :, 3], rv[:, :, 0])  # c4

                        for j in range(4):
                            q = qp * nq + wv * 4 + j
                            # combine: c1*A + c2*att_av + c3*V + c4*att_va
                            o1 = outp.tile([128, D], f32, tag="o1", name=f"o1_{i}_{q}")
                            t_a = outp.tile([128, D], f32, tag="ta", name=f"ta_{i}_{q}")
                            nc.vector.tensor_scalar_mul(o1, aext_t[:, q, 0:D], c1[:, q : q + 1])
                            nc.vector.tensor_scalar_mul(t_a, att_av[:, q, 0:D], c2[:, q : q + 1])
                            nc.vector.tensor_add(o1, o1, t_a)
                            t_b = outp.tile([128, D], f32, tag="tb", name=f"tb_{i}_{q}")
                            nc.vector.tensor_scalar_mul(t_b, vext_t[:, q, 0:D], rv[:, j, 2:3])
                            t_c = outp.tile([128, D], f32, tag="tc", name=f"tc_{i}_{q}")
                            nc.vector.tensor_scalar_mul(t_c, att_va[:, q, 0:D], rv[:, j, 3:4])
                            nc.vector.tensor_add(t_b, t_b, t_c)
                            nc.vector.tensor_add(o1, o1, t_b)
                            nc.sync.dma_start(out=outv[:, q, :], in_=o1)

    nc.compile()
    return nc


def _host_prep(A, V, W_A_g, W_V_g, b_A_g, b_V_g):
    import jax

    if "eps" not in _cache:
        fn = jax.jit(
            lambda: (
                jax.random.normal(jax.random.key(42), (B, T, D), dtype=np.float32),
                jax.random.normal(jax.random.key(43), (B, T, D), dtype=np.float32),
            ),
            backend="cpu",
        )
        ev, ea = fn()
        _cache["eps"] = (np.asarray(ev), np.asarray(ea))
    eps_v, eps_a = _cache["eps"]

    def resample_T(X, eps):
        mu = X.mean(axis=1, dtype=np.float32)
        Xc = X - mu[:, None, :]
        Sigma = np.matmul(Xc.transpose(0, 2, 1), Xc) / np.float32(T - 1)
        Sigma = Sigma + (1e-6 * np.eye(D, dtype=np.float32))[None]
        L = np.linalg.cholesky(Sigma).astype(np.float32)
        KTr = np.matmul(L, eps.transpose(0, 2, 1)) + mu[:, :, None]
        return np.ascontiguousarray(KTr.astype(np.float32))  # [B, D, T]

    KvT = resample_T(V, eps_v)
    KaT = resample_T(A, eps_a)

    wA1, wA2 = W_A_g[0, :D], W_A_g[0, D:]
    wV1, wV2 = W_V_g[0, :D], W_V_g[0, D:]
    ones = np.ones((B, T, 1), np.float32)
    v_g = (V @ wA2)[..., None]
    a_g = (A @ wV2)[..., None]

    if P_DT == "fp16":
        vg_hi = v_g.astype(np.float16).astype(np.float32)
        ag_hi = a_g.astype(np.float16).astype(np.float32)
        zeros = np.zeros((B, T, 1), np.float32)
        vext = np.concatenate([V, vg_hi, v_g - vg_hi, ones, zeros], axis=2).astype(np.float16)
        aext = np.concatenate([A, ag_hi, a_g - ag_hi, ones, zeros], axis=2).astype(np.float16)
    else:
        zeros = np.zeros((B, T, 1), np.float32)
        vext = np.concatenate([V, v_g, zeros, ones, zeros], axis=2).astype(np.float32)
        aext = np.concatenate([A, a_g, zeros, ones, zeros], axis=2).astype(np.float32)

    # host-side row stats: gate row-dots and the cosine gate alpha
    a_glin = np.einsum("btd,d->bt", A, wA1).astype(np.float32)
    v_glin = np.einsum("btd,d->bt", V, wV1).astype(np.float32)
    dot = np.sum(A * V, axis=2)
    nrm = np.sqrt(np.sum(A * A, axis=2)) * np.sqrt(np.sum(V * V, axis=2))
    cos = dot / np.maximum(nrm, 1e-8)
    alpha = (1.0 / (1.0 + np.exp(-cos))).astype(np.float32)

    bg = np.array([[b_A_g[0], b_V_g[0]]], dtype=np.float32)

    AT = np.ascontiguousarray(A.transpose(0, 2, 1))
    VT = np.ascontiguousarray(V.transpose(0, 2, 1))
    vext = np.ascontiguousarray(vext)
    aext = np.ascontiguousarray(aext)

    in_maps = []
    for c in range(NCORES):
        m = {"bg": bg}
        for i in range(BPC):
            b = c * BPC + i
            m[f"at{i}"] = AT[b]
            m[f"vt{i}"] = VT[b]
            m[f"kvt{i}"] = KvT[b]
            m[f"kat{i}"] = KaT[b]
            m[f"aext{i}"] = aext[b]
            m[f"vext{i}"] = vext[b]
            m[f"alp{i}"] = np.ascontiguousarray(alpha[b])
            m[f"agl{i}"] = np.ascontiguousarray(a_glin[b])
            m[f"vgl{i}"] = np.ascontiguousarray(v_glin[b])
        in_maps.append(m)
    return in_maps


def kernel(A, V, W_A_g, W_V_g, b_A_g, b_V_g):
    from concourse import bass_utils

    A = np.asarray(A, dtype=np.float32)
    V = np.asarray(V, dtype=np.float32)
    W_A_g = np.asarray(W_A_g, dtype=np.float32)
    W_V_g = np.asarray(W_V_g, dtype=np.float32)
    b_A_g = np.asarray(b_A_g, dtype=np.float32)
    b_V_g = np.asarray(b_V_g, dtype=np.float32)

    if "nc" not in _cache:
        _cache["nc"] = _build()
    nc = _cache["nc"]

    in_maps = _host_prep(A, V, W_A_g, W_V_g, b_A_g, b_V_g)
    res = bass_utils.run_bass_kernel_spmd(nc, in_maps, core_ids=list(range(NCORES)))

    out = np.empty((B, T, D), np.float32)
    for c in range(NCORES):
        for i in range(BPC):
            out[c * BPC + i] = res.results[c][f"out{i}"]
    return out
